# revision 1
# baseline (speedup 1.0000x reference)
"""Trainium2 Bass kernel for nn_HFMelSpectrogram.

Pipeline (per core, 4 batches of the 32-batch waveform):
  1. STFT-as-GEMM: spec[i, t] = sum_n Wp[i, n] * x[480*t + n], n in [0,1024).
     Host packs the 1024 nontrivial DFT rows (513 cos + 511 sin; the sin rows
     for k=0 and k=512 are identically zero) so the contraction is exactly
     8 x 128.  The frame matrix is supplied as two phase-shifted block
     matrices XtA[j,t] = x[480t+j], XtB[j,t] = x[480t+512+j] (j < 512) so the
     rhs operand needs no on-chip transpose or column shifting.
  2. Square on ScalarE (PSUM -> SBUF, bf16).
  3. Mel projection: melT[t, m] = sum_i sq[i, t] * Mexp[i, m] where Mexp maps
     each packed row back to its mel filter column (power = cos^2 + sin^2 is
     absorbed into the GEMM).  bf16, free dim = 64.
  4. Ln on ScalarE -> logmelT[t, m] (the 10/log(10) scale is folded into R).
  5. Bilinear height-resize 1000 -> 1024 as a banded GEMM: each 128-row
     h-tile draws from at most two 128-row t-tiles.  The 4 batches are packed
     side by side in the free dim (4*64 = 256) so fp32r runs at full rate.
All large matmuls use float32r (full-rate fp32 path on TRN2).
"""

import numpy as np
import ml_dtypes

import concourse.bass as bass
import concourse.bacc as bacc
import concourse.tile as tile
import concourse.mybir as mybir
from concourse.bass_utils import run_bass_kernel_spmd

F32 = mybir.dt.float32
F32R = mybir.dt.float32r
BF16 = mybir.dt.bfloat16

N_FFT = 1024
HOP = 480
NB_MAX = 1000      # frames kept by the reference
N_MELS = 64
SPECW = 1024       # output height after resize
NBINS = 513
B, L = 32, 480000
NCORES = 8
BPC = B // NCORES  # batches per core
TFR = 1024         # padded frame count (frames >= 1000 are zeroed via R)
PAD = N_FFT // 2

# Set by test harness to collect a profile; harness default leaves it off.
TRACE = False
LAST_RESULTS = None


def _resize_blocks():
    """Banded resize matrix blocks, f32 coords matching the reference."""
    scale = np.float32((NB_MAX - 1) / (SPECW - 1))
    pos = np.arange(SPECW, dtype=np.float32) * scale
    h0 = np.clip(np.floor(pos).astype(np.int64), 0, NB_MAX - 1)
    frac = (pos - h0.astype(np.float32)).astype(np.float64)
    h1 = np.minimum(h0 + 1, NB_MAX - 1)
    c = 10.0 / np.log(10.0)
    r = np.zeros((TFR, SPECW), np.float64)
    cols = np.arange(SPECW)
    r[h0, cols] += (1.0 - frac) * c
    r[h1, cols] += frac * c
    blocks = np.zeros((8, 2, 128, 128), np.float32)
    qpairs = []
    for g in range(8):
        sub = r[:, g * 128:(g + 1) * 128]
        rows = np.nonzero(sub.any(axis=1))[0]
        qs = sorted({int(q) for q in rows // 128})
        assert 1 <= len(qs) <= 2, qs
        q0 = qs[0]
        q1 = qs[1] if len(qs) > 1 else min(q0 + 1, 7)
        blocks[g, 0] = sub[q0 * 128:(q0 + 1) * 128].astype(np.float32)
        blocks[g, 1] = sub[q1 * 128:(q1 + 1) * 128].astype(np.float32)
        qpairs.append((q0, q1))
    return blocks, qpairs


_RBLOCKS, _QPAIRS = _resize_blocks()


def _build_bass():
    nc = bacc.Bacc("TRN2", target_bir_lowering=False, debug=False,
                   num_devices=NCORES)
    xa = nc.declare_dram_parameter("xa", [BPC, 4, 128, TFR], F32R, isOutput=False)
    xb = nc.declare_dram_parameter("xb", [BPC, 4, 128, TFR], F32R, isOutput=False)
    wt = nc.declare_dram_parameter("wt", [8, 128, 1024], F32R, isOutput=False)
    mexp = nc.declare_dram_parameter("mexp", [8, 128, N_MELS], BF16, isOutput=False)
    rblk = nc.declare_dram_parameter("rblk", [8, 2, 128, 128], F32R, isOutput=False)
    out = nc.declare_dram_parameter("out", [BPC, SPECW, N_MELS], F32, isOutput=True)

    with tile.TileContext(nc) as tc:
        with (
            tc.tile_pool(name="consts", bufs=1) as consts,
            tc.tile_pool(name="xt", bufs=3) as xpool,
            tc.tile_pool(name="sq", bufs=3) as sqpool,
            tc.tile_pool(name="lm", bufs=1) as lmpool,
            tc.tile_pool(name="ot", bufs=3) as otpool,
            tc.tile_pool(name="specp", bufs=5, space="PSUM") as specp,
            tc.tile_pool(name="melp", bufs=2, space="PSUM") as melp,
            tc.tile_pool(name="resp", bufs=1, space="PSUM") as resp,
        ):
            wt_t = []
            for c in range(8):
                t = consts.tile([128, 1024], F32R, tag=f"wt{c}", name=f"wt{c}")
                nc.gpsimd.dma_start(out=t, in_=wt[c])
                wt_t.append(t)
            mexp_t = []
            for c in range(8):
                t = consts.tile([128, N_MELS], BF16, tag=f"me{c}", name=f"me{c}")
                nc.gpsimd.dma_start(out=t, in_=mexp[c])
                mexp_t.append(t)
            rb_t = []
            for g in range(8):
                pair = []
                for j in range(2):
                    t = consts.tile([128, 128], F32R, tag=f"rb{g}_{j}",
                                    name=f"rb{g}_{j}")
                    nc.gpsimd.dma_start(out=t, in_=rblk[g, j])
                    pair.append(t)
                rb_t.append(pair)
            eps_t = consts.tile([128, 1], F32, tag="eps", name="eps")
            nc.vector.memset(eps_t, 1e-10)

            # logmelT for all 4 batches: [t_local, t_chunk g, 4*64]
            logmel = lmpool.tile([128, 8, BPC * N_MELS], F32R, tag="lm",
                                 name="logmel")

            def issue_mel(sq_tiles, b, tt):
                for s in range(4):
                    g = tt * 4 + s
                    mt = melp.tile([128, N_MELS], F32, tag="mel", name="melpsum")
                    for i in range(8):
                        nc.tensor.matmul(
                            mt,
                            lhsT=sq_tiles[i][:, s * 128:(s + 1) * 128],
                            rhs=mexp_t[i],
                            start=(i == 0),
                            stop=(i == 7),
                        )
                    nc.scalar.activation(
                        out=logmel[:, g, b * N_MELS:(b + 1) * N_MELS],
                        in_=mt,
                        func=mybir.ActivationFunctionType.Ln,
                        bias=eps_t,
                        scale=1.0,
                    )

            pending = None
            for b in range(BPC):
                xt = []
                for c in range(8):
                    t = xpool.tile([128, TFR], F32R, tag=f"xt{c}", name=f"xt{c}")
                    src = xa[b, c] if c < 4 else xb[b, c - 4]
                    nc.sync.dma_start(out=t, in_=src)
                    xt.append(t)
                for tt in range(2):
                    sq_tiles = []
                    for m in range(8):
                        ps = specp.tile([128, 512], F32, tag="spec",
                                        name="specpsum")
                        for c in range(8):
                            nc.tensor.matmul(
                                ps,
                                lhsT=wt_t[c][:, m * 128:(m + 1) * 128],
                                rhs=xt[c][:, tt * 512:(tt + 1) * 512],
                                start=(c == 0),
                                stop=(c == 7),
                            )
                        sq = sqpool.tile([128, 512], BF16, tag=f"sq{m}",
                                         name=f"sq{m}")
                        nc.scalar.square(sq, ps)
                        sq_tiles.append(sq)
                    if pending is not None:
                        issue_mel(*pending)
                    pending = (sq_tiles, b, tt)
            issue_mel(*pending)

            for g in range(8):
                q0, q1 = _QPAIRS[g]
                rp = resp.tile([128, BPC * N_MELS], F32, tag="res", name="respsum")
                nc.tensor.matmul(rp, lhsT=rb_t[g][0],
                                 rhs=logmel[:, q0, :],
                                 start=True, stop=False)
                nc.tensor.matmul(rp, lhsT=rb_t[g][1],
                                 rhs=logmel[:, q1, :],
                                 start=False, stop=True)
                ot = otpool.tile([128, BPC * N_MELS], F32, tag="ot", name="ot")
                nc.vector.tensor_copy(out=ot, in_=rp)
                for b in range(BPC):
                    nc.sync.dma_start(
                        out=out[b, g * 128:(g + 1) * 128, :],
                        in_=ot[:, b * N_MELS:(b + 1) * N_MELS],
                    )
    return nc


def _host_prep(waveform, stft_weights, mel_filters):
    wv = np.ascontiguousarray(waveform, dtype=np.float32)
    xp = np.pad(wv, ((0, 0), (PAD, PAD)), mode="reflect")  # [B, 481024]
    need = HOP * (TFR - 1) + 512 + 512  # max index reached by XtB + 1
    xz = np.zeros((B, need), np.float32)
    xz[:, : xp.shape[1]] = xp
    sb = xz.strides[0]
    xta = np.lib.stride_tricks.as_strided(
        xz, shape=(B, 512, TFR), strides=(sb, 4, HOP * 4))
    xtb = np.lib.stride_tricks.as_strided(
        xz[:, 512:], shape=(B, 512, TFR), strides=(sb, 4, HOP * 4))
    xta = np.ascontiguousarray(xta).reshape(B, 4, 128, TFR)
    xtb = np.ascontiguousarray(xtb).reshape(B, 4, 128, TFR)

    w = np.ascontiguousarray(stft_weights, dtype=np.float32)  # [1026, 1024]
    rows = list(range(0, NBINS)) + list(range(NBINS + 1, NBINS + 512))
    assert len(rows) == 1024
    wp = w[rows]                                   # [1024 packed bins, 1024 n]
    wtile = np.ascontiguousarray(wp.T).reshape(8, 128, 1024)

    mf = np.ascontiguousarray(mel_filters, dtype=np.float32)  # [513, 64]
    f_of_i = np.array([i if i < NBINS else i - 512 for i in range(1024)])
    mexp = mf[f_of_i].astype(ml_dtypes.bfloat16).reshape(8, 128, N_MELS)
    return xta, xtb, wtile, mexp


def kernel(waveform, stft_weights, mel_filters):
    global LAST_RESULTS
    xta, xtb, wtile, mexp = _host_prep(waveform, stft_weights, mel_filters)
    nc = _build_bass()
    in_maps = []
    for i in range(NCORES):
        in_maps.append({
            "xa": np.ascontiguousarray(xta[i * BPC:(i + 1) * BPC]),
            "xb": np.ascontiguousarray(xtb[i * BPC:(i + 1) * BPC]),
            "wt": wtile,
            "mexp": mexp,
            "rblk": _RBLOCKS,
        })
    nc.compile()
    res = run_bass_kernel_spmd(nc, in_maps, list(range(NCORES)), trace=TRACE)
    LAST_RESULTS = res
    out = np.concatenate([r["out"] for r in res.results], axis=0)
    return out.reshape(B, 1, SPECW, N_MELS).astype(np.float32)



# revision 11
# speedup vs baseline: 2.2818x; 2.2818x over previous
"""Trainium2 Bass kernel for nn_HFMelSpectrogram.

Pipeline (per core, 4 batches of the 32-batch waveform):
  1. STFT-as-GEMM in fp8e4m3 with DoubleRow perf mode: the PE array is
     virtualized to 128x256, so the 1024-deep contraction takes 4 matmuls
     (each pairs two 128-sample chunks in the [128, 2, N] slot layout).
     Host packs the 1024 nontrivial DFT rows (513 cos + 511 sin) so the
     contraction is exactly 1024; the frame matrix x[480t + n] is shipped
     pre-quantized to fp8 (final rel err ~5e-4, tolerance 2e-2).
  2. Square PSUM -> SBUF fp8 on ScalarE with scale 1/16 (sq = (X/16)^2).
  3. Mel projection in fp8 DoubleRow as well; mel filters are scaled by 64
     so their 0..0.02 range is representable; the net (1/256)*64 = 1/4
     factor is undone by the Ln activation's input scale of 4.
  4. Ln on ScalarE -> logmelT[t, m] bf16 (10/log(10) folded into R).
  5. Bilinear height-resize 1000 -> 1024 as a banded GEMM per batch
     (bf16 weights), overlapped with the next batch's STFT.
"""

import numpy as np
import ml_dtypes

import concourse.bass as bass
import concourse.bacc as bacc
import concourse.tile as tile
import concourse.mybir as mybir
from concourse.bass_utils import run_bass_kernel_spmd

F32 = mybir.dt.float32
BF16 = mybir.dt.bfloat16
FP8 = mybir.dt.float8e4
NP_FP8 = ml_dtypes.float8_e4m3
DR = mybir.MatmulPerfMode.DoubleRow

N_FFT = 1024
HOP = 480
NB_MAX = 1000      # frames kept by the reference
N_MELS = 64
SPECW = 1024       # output height after resize
NBINS = 513
B, L = 32, 480000
NCORES = 8
BPC = B // NCORES  # batches per core
TFR = 1024         # padded frame count (frames >= 1000 are zeroed via R)
PAD = N_FFT // 2

# Set by test harness to collect a profile; harness default leaves it off.
TRACE = False
LAST_RESULTS = None


def _resize_blocks():
    """Banded resize matrix blocks, f32 coords matching the reference."""
    scale = np.float32((NB_MAX - 1) / (SPECW - 1))
    pos = np.arange(SPECW, dtype=np.float32) * scale
    h0 = np.clip(np.floor(pos).astype(np.int64), 0, NB_MAX - 1)
    frac = (pos - h0.astype(np.float32)).astype(np.float64)
    h1 = np.minimum(h0 + 1, NB_MAX - 1)
    c = 10.0 / np.log(10.0)
    r = np.zeros((TFR, SPECW), np.float64)
    cols = np.arange(SPECW)
    r[h0, cols] += (1.0 - frac) * c
    r[h1, cols] += frac * c
    blocks = np.zeros((8, 2, 128, 128), np.float32)
    qpairs = []
    for g in range(8):
        sub = r[:, g * 128:(g + 1) * 128]
        rows = np.nonzero(sub.any(axis=1))[0]
        qs = sorted({int(q) for q in rows // 128})
        assert 1 <= len(qs) <= 2, qs
        q0 = qs[0]
        q1 = qs[1] if len(qs) > 1 else min(q0 + 1, 7)
        blocks[g, 0] = sub[q0 * 128:(q0 + 1) * 128].astype(np.float32)
        blocks[g, 1] = sub[q1 * 128:(q1 + 1) * 128].astype(np.float32)
        qpairs.append((q0, q1))
    return blocks.astype(ml_dtypes.bfloat16), qpairs


_RBLOCKS, _QPAIRS = _resize_blocks()


def _build_bass():
    nc = bacc.Bacc("TRN2", target_bir_lowering=False, debug=False,
                   num_devices=NCORES)
    xt8 = nc.declare_dram_parameter("xt8", [BPC, 4, 128, 2, TFR], FP8,
                                    isOutput=False)
    wt = nc.declare_dram_parameter("wt", [4, 128, 2, 1024], FP8, isOutput=False)
    mexp = nc.declare_dram_parameter("mexp", [4, 128, 2, N_MELS], FP8,
                                     isOutput=False)
    rblk = nc.declare_dram_parameter("rblk", [8, 2, 128, 128], BF16,
                                     isOutput=False)
    out = nc.declare_dram_parameter("out", [BPC, SPECW, N_MELS], F32,
                                    isOutput=True)

    with tile.TileContext(nc) as tc:
        with (
            tc.tile_pool(name="consts", bufs=1) as consts,
            tc.tile_pool(name="xt", bufs=3) as xpool,
            tc.tile_pool(name="sq", bufs=3) as sqpool,
            tc.tile_pool(name="lm", bufs=2) as lmpool,
            tc.tile_pool(name="ot", bufs=4) as otpool,
            tc.tile_pool(name="specp", bufs=4, space="PSUM") as specp,
            tc.tile_pool(name="melp", bufs=2, space="PSUM") as melp,
            tc.tile_pool(name="resp", bufs=2, space="PSUM") as resp,
        ):
            wt_t = []
            for p in range(4):
                t = consts.tile([128, 2, 1024], FP8, tag=f"wt{p}", name=f"wt{p}")
                nc.gpsimd.dma_start(out=t, in_=wt[p])
                wt_t.append(t)
            mexp_t = []
            for p in range(4):
                t = consts.tile([128, 2, N_MELS], FP8, tag=f"me{p}",
                                name=f"me{p}")
                nc.gpsimd.dma_start(out=t, in_=mexp[p])
                mexp_t.append(t)
            rb_t = []
            for g in range(8):
                pair = []
                for j in range(2):
                    t = consts.tile([128, 128], BF16, tag=f"rb{g}_{j}",
                                    name=f"rb{g}_{j}")
                    nc.gpsimd.dma_start(out=t, in_=rblk[g, j])
                    pair.append(t)
                rb_t.append(pair)
            eps_t = consts.tile([128, 1], F32, tag="eps", name="eps")
            nc.vector.memset(eps_t, 1e-10)

            def issue_mel(sq_tiles, logmel, tt):
                for s in range(4):
                    g = tt * 4 + s
                    mt = melp.tile([128, N_MELS], F32, tag="mel", name="melpsum")
                    for p in range(4):
                        nc.tensor.matmul(
                            mt,
                            lhsT=sq_tiles[p][:, :, s * 128:(s + 1) * 128],
                            rhs=mexp_t[p],
                            start=(p == 0),
                            stop=(p == 3),
                            perf_mode=DR,
                        )
                    nc.scalar.activation(
                        out=logmel[:, g, :],
                        in_=mt,
                        func=mybir.ActivationFunctionType.Ln,
                        bias=eps_t,
                        scale=4.0,
                    )

            def issue_resize(logmel, b):
                for g in range(8):
                    q0, q1 = _QPAIRS[g]
                    rp = resp.tile([128, N_MELS], F32, tag="res", name="respsum")
                    nc.tensor.matmul(rp, lhsT=rb_t[g][0], rhs=logmel[:, q0, :],
                                     start=True, stop=False)
                    nc.tensor.matmul(rp, lhsT=rb_t[g][1], rhs=logmel[:, q1, :],
                                     start=False, stop=True)
                    ot = otpool.tile([128, N_MELS], F32, tag="ot", name="ot")
                    nc.vector.tensor_copy(out=ot, in_=rp)
                    nc.sync.dma_start(out=out[b, g * 128:(g + 1) * 128, :],
                                      in_=ot)

            pending_mel = None
            pending_res = None
            for b in range(BPC):
                xt = []
                for p in range(4):
                    t = xpool.tile([128, 2, TFR], FP8, tag=f"xt{p}",
                                   name=f"xt{p}")
                    nc.sync.dma_start(out=t, in_=xt8[b, p])
                    xt.append(t)
                logmel = lmpool.tile([128, 8, N_MELS], BF16, tag="lm",
                                     name="logmel")
                for tt in range(2):
                    sq_tiles = []
                    for m in range(8):
                        ps = specp.tile([128, 512], F32, tag="spec",
                                        name="specpsum")
                        for p in range(4):
                            nc.tensor.matmul(
                                ps,
                                lhsT=wt_t[p][:, :, m * 128:(m + 1) * 128],
                                rhs=xt[p][:, :, tt * 512:(tt + 1) * 512],
                                start=(p == 0),
                                stop=(p == 3),
                                perf_mode=DR,
                            )
                        if m % 2 == 0:
                            sq = sqpool.tile([128, 2, 512], FP8,
                                             tag=f"sq{m // 2}",
                                             name=f"sq{m // 2}")
                            sq_tiles.append(sq)
                        nc.scalar.activation(
                            out=sq_tiles[m // 2][:, m % 2, :],
                            in_=ps,
                            func=mybir.ActivationFunctionType.Square,
                            bias=0.0,
                            scale=0.0625,
                        )
                    if pending_mel is not None:
                        issue_mel(*pending_mel)
                    pending_mel = (sq_tiles, logmel, tt)
                    if pending_res is not None:
                        issue_resize(*pending_res)
                        pending_res = None
                pending_res = (logmel, b)
            issue_mel(*pending_mel)
            issue_resize(*pending_res)
    return nc


def _host_prep(waveform, stft_weights, mel_filters):
    wv = np.ascontiguousarray(waveform, dtype=np.float32)
    xp = np.pad(wv, ((0, 0), (PAD, PAD)), mode="reflect")  # [B, 481024]
    need = HOP * (TFR - 1) + N_FFT  # max index reached by a frame + 1
    xz = np.zeros((B, need), NP_FP8)
    xz[:, : xp.shape[1]] = xp.astype(NP_FP8)
    sb = xz.strides[0]
    # xt8[b, p, s, j, t] = x[480t + 256p + 128s + j] -> transpose to
    # [b, p, j, s, t] for the DoubleRow [128, 2, N] slot layout.
    xv = np.lib.stride_tricks.as_strided(
        xz, shape=(B, 4, 2, 128, TFR), strides=(sb, 256, 128, 1, HOP))
    xt8 = np.ascontiguousarray(xv.transpose(0, 1, 3, 2, 4))
    xt8 = xt8.reshape(B, 4, 128, 2, TFR)

    w = np.ascontiguousarray(stft_weights, dtype=np.float32)  # [1026, 1024]
    rows = list(range(0, NBINS)) + list(range(NBINS + 1, NBINS + 512))
    assert len(rows) == 1024
    wp = w[rows]                                   # [1024 packed bins, 1024 n]
    # wt[p, j, s, k] = W[k, 256p + 128s + j]
    wt = wp.T.reshape(4, 2, 128, 1024).transpose(0, 2, 1, 3)
    wt = np.ascontiguousarray(wt).astype(NP_FP8)

    mf = np.ascontiguousarray(mel_filters, dtype=np.float32)  # [513, 64]
    f_of_i = np.array([i if i < NBINS else i - 512 for i in range(1024)])
    # mexp[p, j, s, m] = 64 * mf[bin(256p + 128s + j), m]; the 64 and the
    # 1/256 from sq = (X/16)^2 are undone by the Ln input scale of 4.
    mexp = (64.0 * mf[f_of_i]).reshape(4, 2, 128, N_MELS)
    mexp = np.ascontiguousarray(mexp.transpose(0, 2, 1, 3)).astype(NP_FP8)
    return xt8, wt, mexp


def kernel(waveform, stft_weights, mel_filters):
    global LAST_RESULTS
    xt8, wt, mexp = _host_prep(waveform, stft_weights, mel_filters)
    nc = _build_bass()
    in_maps = []
    for i in range(NCORES):
        in_maps.append({
            "xt8": np.ascontiguousarray(xt8[i * BPC:(i + 1) * BPC]),
            "wt": wt,
            "mexp": mexp,
            "rblk": _RBLOCKS,
        })
    nc.compile()
    res = run_bass_kernel_spmd(nc, in_maps, list(range(NCORES)), trace=TRACE)
    LAST_RESULTS = res
    out = np.concatenate([r["out"] for r in res.results], axis=0)
    return out.reshape(B, 1, SPECW, N_MELS).astype(np.float32)


# revision 15
# speedup vs baseline: 2.3565x; 1.0327x over previous
"""Trainium2 Bass kernel for nn_HFMelSpectrogram.

Pipeline (per core, 4 batches of the 32-batch waveform):
  1. STFT-as-GEMM in fp8e4m3 with DoubleRow perf mode: the PE array is
     virtualized to 128x256, so the 1024-deep contraction takes 4 matmuls
     (each pairs two 128-sample chunks in the [128, 2, N] slot layout).
     Host packs the 1024 nontrivial DFT rows (513 cos + 511 sin) so the
     contraction is exactly 1024; the frame matrix x[480t + n] is shipped
     pre-quantized to fp8 (final rel err ~5e-4, tolerance 2e-2).
  2. Square PSUM -> SBUF fp8 on ScalarE with scale 1/16 (sq = (X/16)^2).
  3. Mel projection in fp8 DoubleRow as well; mel filters are scaled by 64
     so their 0..0.02 range is representable; the net (1/256)*64 = 1/4
     factor is undone by the Ln activation's input scale of 4.
  4. Ln on ScalarE -> logmelT[t, m] bf16 (10/log(10) folded into R).
  5. Bilinear height-resize 1000 -> 1024 as a banded GEMM per batch
     (bf16 weights), overlapped with the next batch's STFT.
"""

import numpy as np
import ml_dtypes

import concourse.bass as bass
import concourse.bacc as bacc
import concourse.tile as tile
import concourse.mybir as mybir
from concourse.bass_utils import run_bass_kernel_spmd

F32 = mybir.dt.float32
BF16 = mybir.dt.bfloat16
FP8 = mybir.dt.float8e4
NP_FP8 = ml_dtypes.float8_e4m3
DR = mybir.MatmulPerfMode.DoubleRow

N_FFT = 1024
HOP = 480
NB_MAX = 1000      # frames kept by the reference
N_MELS = 64
SPECW = 1024       # output height after resize
NBINS = 513
B, L = 32, 480000
NCORES = 8
BPC = B // NCORES  # batches per core
TFR = 1024         # padded frame count (frames >= 1000 are zeroed via R)
PAD = N_FFT // 2

# Set by test harness to collect a profile; harness default leaves it off.
TRACE = False
LAST_RESULTS = None


def _resize_blocks():
    """Banded resize matrix blocks, f32 coords matching the reference."""
    scale = np.float32((NB_MAX - 1) / (SPECW - 1))
    pos = np.arange(SPECW, dtype=np.float32) * scale
    h0 = np.clip(np.floor(pos).astype(np.int64), 0, NB_MAX - 1)
    frac = (pos - h0.astype(np.float32)).astype(np.float64)
    h1 = np.minimum(h0 + 1, NB_MAX - 1)
    c = 10.0 / np.log(10.0)
    r = np.zeros((TFR, SPECW), np.float64)
    cols = np.arange(SPECW)
    r[h0, cols] += (1.0 - frac) * c
    r[h1, cols] += frac * c
    blocks = np.zeros((8, 2, 128, 128), np.float32)
    qpairs = []
    for g in range(8):
        sub = r[:, g * 128:(g + 1) * 128]
        rows = np.nonzero(sub.any(axis=1))[0]
        qs = sorted({int(q) for q in rows // 128})
        assert 1 <= len(qs) <= 2, qs
        q0 = qs[0]
        q1 = qs[1] if len(qs) > 1 else min(q0 + 1, 7)
        blocks[g, 0] = sub[q0 * 128:(q0 + 1) * 128].astype(np.float32)
        blocks[g, 1] = sub[q1 * 128:(q1 + 1) * 128].astype(np.float32)
        qpairs.append((q0, q1))
    return blocks.astype(ml_dtypes.bfloat16), qpairs


_RBLOCKS, _QPAIRS = _resize_blocks()


def _build_bass():
    nc = bacc.Bacc("TRN2", target_bir_lowering=False, debug=False,
                   num_devices=NCORES)
    xt8 = nc.declare_dram_parameter("xt8", [BPC, 2, 4, 128, 2, TFR // 2], FP8,
                                    isOutput=False)
    wt = nc.declare_dram_parameter("wt", [4, 128, 2, 1024], FP8, isOutput=False)
    mexp = nc.declare_dram_parameter("mexp", [4, 128, 2, N_MELS], FP8,
                                     isOutput=False)
    rblk = nc.declare_dram_parameter("rblk", [8, 2, 128, 128], BF16,
                                     isOutput=False)
    out = nc.declare_dram_parameter("out", [BPC, SPECW, N_MELS], F32,
                                    isOutput=True)

    with tile.TileContext(nc) as tc:
        with (
            tc.tile_pool(name="consts", bufs=1) as consts,
            tc.tile_pool(name="xt", bufs=3) as xpool,
            tc.tile_pool(name="sq", bufs=3) as sqpool,
            tc.tile_pool(name="lm", bufs=2) as lmpool,
            tc.tile_pool(name="ot", bufs=4) as otpool,
            tc.tile_pool(name="specp", bufs=4, space="PSUM") as specp,
            tc.tile_pool(name="melp", bufs=2, space="PSUM") as melp,
            tc.tile_pool(name="resp", bufs=2, space="PSUM") as resp,
        ):
            wt_t = []
            for p in range(4):
                t = consts.tile([128, 2, 1024], FP8, tag=f"wt{p}", name=f"wt{p}")
                nc.gpsimd.dma_start(out=t, in_=wt[p])
                wt_t.append(t)

            def load_late_consts():
                # issued after batch 0's frame DMAs so they don't delay the
                # first STFT matmul (they're only needed ~10us in)
                mexp_t = []
                for p in range(4):
                    t = consts.tile([128, 2, N_MELS], FP8, tag=f"me{p}",
                                    name=f"me{p}")
                    nc.gpsimd.dma_start(out=t, in_=mexp[p])
                    mexp_t.append(t)
                rb_t = []
                for g in range(8):
                    pair = []
                    for j in range(2):
                        t = consts.tile([128, 128], BF16, tag=f"rb{g}_{j}",
                                        name=f"rb{g}_{j}")
                        nc.gpsimd.dma_start(out=t, in_=rblk[g, j])
                        pair.append(t)
                    rb_t.append(pair)
                return mexp_t, rb_t

            eps_t = consts.tile([128, 1], F32, tag="eps", name="eps")
            nc.vector.memset(eps_t, 1e-10)

            mexp_t = []
            rb_t = []

            def issue_mel(sq_tiles, logmel, tt):
                mt = melp.tile([128, 4 * N_MELS], F32, tag="mel",
                               name="melpsum")
                for s in range(4):
                    for p in range(4):
                        nc.tensor.matmul(
                            mt[:, s * N_MELS:(s + 1) * N_MELS],
                            lhsT=sq_tiles[p][:, :, s * 128:(s + 1) * 128],
                            rhs=mexp_t[p],
                            start=(p == 0),
                            stop=(p == 3),
                            perf_mode=DR,
                        )
                nc.scalar.activation(
                    out=logmel[:, tt * 256:(tt + 1) * 256],
                    in_=mt,
                    func=mybir.ActivationFunctionType.Ln,
                    bias=eps_t,
                    scale=4.0,
                )

            def issue_resize(logmel, b, half):
                for g in range(half * 4, half * 4 + 4):
                    q0, q1 = _QPAIRS[g]
                    rp = resp.tile([128, N_MELS], F32, tag="res", name="respsum")
                    nc.tensor.matmul(rp, lhsT=rb_t[g][0],
                                     rhs=logmel[:, q0 * 64:(q0 + 1) * 64],
                                     start=True, stop=False)
                    nc.tensor.matmul(rp, lhsT=rb_t[g][1],
                                     rhs=logmel[:, q1 * 64:(q1 + 1) * 64],
                                     start=False, stop=True)
                    ot = otpool.tile([128, N_MELS], F32, tag="ot", name="ot")
                    nc.vector.tensor_copy(out=ot, in_=rp)
                    nc.sync.dma_start(out=out[b, g * 128:(g + 1) * 128, :],
                                      in_=ot)

            pending = []
            for b in range(BPC):
                xt = []
                for tt in range(2):
                    row = []
                    for p in range(4):
                        t = xpool.tile([128, 2, TFR // 2], FP8,
                                       tag=f"xt{tt}{p}", name=f"xt{tt}{p}")
                        nc.sync.dma_start(out=t, in_=xt8[b, tt, p])
                        row.append(t)
                    xt.append(row)
                if b == 0:
                    me, rb = load_late_consts()
                    mexp_t.extend(me)
                    rb_t.extend(rb)
                logmel = lmpool.tile([128, 512], BF16, tag="lm", name="logmel")
                for tt in range(2):
                    sq_tiles = []
                    for m in range(8):
                        ps = specp.tile([128, 512], F32, tag="spec",
                                        name="specpsum")
                        for p in range(4):
                            nc.tensor.matmul(
                                ps,
                                lhsT=wt_t[p][:, :, m * 128:(m + 1) * 128],
                                rhs=xt[tt][p],
                                start=(p == 0),
                                stop=(p == 3),
                                perf_mode=DR,
                            )
                        if m % 2 == 0:
                            sq = sqpool.tile([128, 2, 512], FP8,
                                             tag=f"sq{m // 2}",
                                             name=f"sq{m // 2}")
                            sq_tiles.append(sq)
                        nc.scalar.activation(
                            out=sq_tiles[m // 2][:, m % 2, :],
                            in_=ps,
                            func=mybir.ActivationFunctionType.Square,
                            bias=0.0,
                            scale=0.0625,
                        )
                    for fn in pending:
                        fn()
                    pending = [
                        (lambda s=sq_tiles, l=logmel, t=tt: issue_mel(s, l, t)),
                        (lambda l=logmel, bb=b, h=tt: issue_resize(l, bb, h)),
                    ]
            for fn in pending:
                fn()
    return nc


def _host_prep(waveform, stft_weights, mel_filters):
    wv = np.ascontiguousarray(waveform, dtype=np.float32)
    xp = np.pad(wv, ((0, 0), (PAD, PAD)), mode="reflect")  # [B, 481024]
    need = HOP * (TFR - 1) + N_FFT  # max index reached by a frame + 1
    xz = np.zeros((B, need), NP_FP8)
    xz[:, : xp.shape[1]] = xp.astype(NP_FP8)
    sb = xz.strides[0]
    # xt8[b, tt, p, j, s, t'] = x[480(512tt + t') + 256p + 128s + j] --
    # per-(b, tt, p) contiguous tiles in the DoubleRow [128, 2, N] slot
    # layout so each half-batch of frames is one dense DMA.
    xv = np.lib.stride_tricks.as_strided(
        xz, shape=(B, 2, 4, 2, 128, TFR // 2),
        strides=(sb, HOP * (TFR // 2), 256, 128, 1, HOP))
    xt8 = np.ascontiguousarray(xv.transpose(0, 1, 2, 4, 3, 5))

    w = np.ascontiguousarray(stft_weights, dtype=np.float32)  # [1026, 1024]
    rows = list(range(0, NBINS)) + list(range(NBINS + 1, NBINS + 512))
    assert len(rows) == 1024
    wp = w[rows]                                   # [1024 packed bins, 1024 n]
    # wt[p, j, s, k] = W[k, 256p + 128s + j]
    wt = wp.T.reshape(4, 2, 128, 1024).transpose(0, 2, 1, 3)
    wt = np.ascontiguousarray(wt).astype(NP_FP8)

    mf = np.ascontiguousarray(mel_filters, dtype=np.float32)  # [513, 64]
    f_of_i = np.array([i if i < NBINS else i - 512 for i in range(1024)])
    # mexp[p, j, s, m] = 64 * mf[bin(256p + 128s + j), m]; the 64 and the
    # 1/256 from sq = (X/16)^2 are undone by the Ln input scale of 4.
    mexp = (64.0 * mf[f_of_i]).reshape(4, 2, 128, N_MELS)
    mexp = np.ascontiguousarray(mexp.transpose(0, 2, 1, 3)).astype(NP_FP8)
    return xt8, wt, mexp


def kernel(waveform, stft_weights, mel_filters):
    global LAST_RESULTS
    xt8, wt, mexp = _host_prep(waveform, stft_weights, mel_filters)
    nc = _build_bass()
    in_maps = []
    for i in range(NCORES):
        in_maps.append({
            "xt8": np.ascontiguousarray(xt8[i * BPC:(i + 1) * BPC]),
            "wt": wt,
            "mexp": mexp,
            "rblk": _RBLOCKS,
        })
    nc.compile()
    res = run_bass_kernel_spmd(nc, in_maps, list(range(NCORES)), trace=TRACE)
    LAST_RESULTS = res
    out = np.concatenate([r["out"] for r in res.results], axis=0)
    return out.reshape(B, 1, SPECW, N_MELS).astype(np.float32)
